# revision 18
# baseline (speedup 1.0000x reference)
"""DGCNN (dynamic edge conv, B=16 events x N=2048 points, k=20) on 8 TRN2 cores.

Sharding: data-parallel over events. Each core runs an identical Bass program
on 2 events (its slice of pos); MLP weights replicated. Host gathers the
[5, 2] per-core outputs into the final [16, 5].

Device algorithm per event:
  conv1: nu = x_p . x_j - |x_j|^2/2 (PE matmul, order == -dist). Exact top-20
         per row via hierarchical DVE top-k (reduce-max over runs of 8 ->
         top-20 runs -> indirect-DMA drill of the 160 candidate values from
         DRAM -> final top-20 + integer index reconstruction). Edge MLP layer 1
         via A/B split (z_ij = A_i + B_j, bias folded into B); B rows gathered
         point-major by indirect DMA, layers 2/3 run channel-major on PE after
         j-pair PE transposes; max over k fused as PSUM reduce before bias+relu.
  conv2: single edge layer, so max_j relu(A_i+B_j) == relu(A_i + max_j B_j):
         only the B gather is per-edge; max tree on gpsimd.
  lin1 + global max pool folded as reduce-max of PSUM before relu; mlp1/mlp2/
  head on [1024]x2 vectors. BatchNorm affines folded into following layers on
  the host (valid for gamma >= 0, exact here).
"""

from contextlib import ExitStack

import numpy as np

import concourse.bacc as bacc
import concourse.bass as bass
import concourse.mybir as mybir
from concourse.bass import IndirectOffsetOnAxis
from concourse.tile import TileContext

F32 = mybir.dt.float32
U32 = mybir.dt.uint32
AF = mybir.ActivationFunctionType
ALU = mybir.AluOpType
AX = mybir.AxisListType

B, N, K = 16, 2048, 20
NCORES = 8
E = B // NCORES          # events per core
P = 128                  # partition rows per block
NB = N // P              # 16 blocks per event
RUN = 8                  # drill run length
NRUNS = N // RUN         # 256
NCHUNK = 512             # matmul moving free dim
NEG = -3.0e38


# ---------------------------------------------------------------- host prep

def _np(x):
    return np.asarray(x)


def fold_params(params):
    """Fold each Linear->ReLU->BN(eval affine) chain so the device only does
    Linear(+bias) -> (pool) -> ReLU, with the affine folded into the next
    linear. Returns the flat dict of host arrays shipped to every core."""
    c1 = [tuple(map(_np, l)) for l in params["conv1"]]
    c2 = [tuple(map(_np, l)) for l in params["conv2"]]
    l1 = tuple(map(_np, params["lin1"][0]))
    m1 = tuple(map(_np, params["mlp1"][0]))
    m2 = tuple(map(_np, params["mlp2"][0]))
    hW, hb = _np(params["headW"]), _np(params["headb"])

    out = {}
    # conv1 layer 1: z = x_i@(Wt-Wb) + x_j@Wb + b ; bias carried inside B.
    W, b, g1, be1 = c1[0]
    Wt, Wb = W[:3], W[3:]
    out["rA1"] = (Wt - Wb).astype(np.float32)                      # [3, 64]
    out["rB1"] = np.concatenate([Wb, b[None, :]], 0).astype(np.float32)  # [4, 64]
    # conv1 layer 2/3 with previous affine folded in.
    W2, b2, g2, be2 = c1[1]
    W2f_ = (g1[:, None] * W2).astype(np.float32)
    out["W2f"] = np.concatenate([W2f_, W2f_], axis=0)              # [128, 64] x2
    out["b2f"] = (b2 + be1 @ W2).astype(np.float32)[:, None]       # [64, 1]
    W3, b3, g3, be3 = c1[2]
    out["W3f"] = (g2[:, None] * W3).astype(np.float32)
    out["b3f"] = (b3 + be2 @ W3).astype(np.float32)[:, None]
    out["g3"] = g3.astype(np.float32)[:, None]                     # [64, 1]
    out["be3"] = be3.astype(np.float32)[:, None]
    # conv2 single layer on true x1 (affine g3/be3 applied on device).
    Wc, bc, gc, bec = c2[0]
    Wct, Wcb = Wc[:64], Wc[64:]
    out["rA2"] = (Wct - Wcb).astype(np.float32)                    # [64, 128]
    out["rB2"] = np.concatenate([Wcb, bc[None, :]], 0).astype(np.float32)  # [65,128]
    out["gc2"] = gc.astype(np.float32)[:, None]                    # [128, 1]
    out["bec2"] = bec.astype(np.float32)[:, None]
    # lin1 on true [x1; x2].
    Wl, bl, gl, bel = l1
    out["Wl1a"] = Wl[:64].astype(np.float32)                       # [64, 1024]
    out["Wl1b"] = Wl[64:].astype(np.float32)                       # [128, 1024]
    out["bl1"] = bl.reshape(8, 128).T.astype(np.float32)           # [128, 8] col m
    # mlp1 with lin1 affine folded (valid: pool commutes since gl >= 0).
    Wm1, bm1, gm1, bem1 = m1
    Wm1f = (gl[:, None] * Wm1).astype(np.float32)                  # [1024, 512]
    bm1f = (bm1 + bel @ Wm1).astype(np.float32)
    # [1024,512] -> [128, 8*512]: col block k holds rows k*128:(k+1)*128.
    out["Wm1s"] = np.concatenate([Wm1f[k * 128:(k + 1) * 128] for k in range(8)],
                                 axis=1).astype(np.float32)        # [128, 4096]
    out["bm1f"] = bm1f.reshape(4, 128).T.astype(np.float32)        # [128, 4] col m
    Wm2, bm2, gm2, bem2 = m2
    Wm2f = (gm1[:, None] * Wm2).astype(np.float32)                 # [512, 256]
    bm2f = (bm2 + bem1 @ Wm2).astype(np.float32)
    out["Wm2s"] = np.concatenate([Wm2f[k * 128:(k + 1) * 128] for k in range(4)],
                                 axis=1).astype(np.float32)        # [128, 1024]
    out["bm2f"] = bm2f.reshape(2, 128).T.astype(np.float32)        # [128, 2]
    Whf = (gm2[:, None] * hW).astype(np.float32)                   # [256, 5]
    bhf = (hb + bem2 @ hW).astype(np.float32)
    out["Whs"] = np.concatenate([Whf[k * 128:(k + 1) * 128] for k in range(2)],
                                 axis=1).astype(np.float32)        # [128, 10]
    out["bhf"] = bhf.astype(np.float32)[:, None]                   # [5, 1]
    return out


WEIGHT_SPECS = [
    ("rA1", [3, 64]), ("rB1", [4, 64]),
    ("W2f", [128, 64]), ("b2f", [64, 1]),
    ("W3f", [64, 64]), ("b3f", [64, 1]),
    ("g3", [64, 1]), ("be3", [64, 1]),
    ("rA2", [64, 128]), ("rB2", [65, 128]),
    ("gc2", [128, 1]), ("bec2", [128, 1]),
    ("Wl1a", [64, 1024]), ("Wl1b", [128, 1024]), ("bl1", [128, 8]),
    ("Wm1s", [128, 4096]), ("bm1f", [128, 4]),
    ("Wm2s", [128, 1024]), ("bm2f", [128, 2]),
    ("Whs", [128, 10]), ("bhf", [5, 1]),
    ("ident", [128, 128]),
    ("row1", [1, 2048]),
]


# ------------------------------------------------------------ device program

def _topk20_idx(nc, sp, nus):
    """Exact per-row top-20 column indices of nus [128, 2048]: 3x max8 +
    2x match_replace + 3x max_index full scans. Returns idx [128,24] u32
    (first 20 valid)."""
    nb = sp.tile([P, N], F32, tag="nb", bufs=1)
    nc2 = sp.tile([P, N], F32, tag="nc2", bufs=1)
    t8 = sp.tile([P, 24], F32, tag="t8")
    i3 = sp.tile([P, 24], U32, tag="i3")
    nc.vector.max(out=t8[:, 0:8], in_=nus[:])
    nc.vector.match_replace(out=nb[:], in_to_replace=t8[:, 0:8], in_values=nus[:],
                            imm_value=NEG)
    nc.vector.max(out=t8[:, 8:16], in_=nb[:])
    nc.vector.match_replace(out=nc2[:], in_to_replace=t8[:, 8:16], in_values=nb[:],
                            imm_value=NEG)
    nc.vector.max(out=t8[:, 16:24], in_=nc2[:])
    nc.vector.max_index(out=i3[:, 0:8], in_max=t8[:, 0:8], in_values=nus[:])
    nc.vector.max_index(out=i3[:, 8:16], in_max=t8[:, 8:16], in_values=nus[:])
    nc.vector.max_index(out=i3[:, 16:24], in_max=t8[:, 16:24], in_values=nus[:])
    return i3


def _idx_to_gather_list(nc, sp, ptp, ident, idx24):
    """idx24 [128, 24] u32 (cols 0:20 = point ids) -> int16 index list tile
    [16, 160] in dma_gather wrapped layout: flat[i] at [i%16, i//16] with
    flat[g*128+p] = idx24[p, g]."""
    idf = sp.tile([P, K], F32, tag="idf")
    nc.vector.tensor_copy(out=idf[:], in_=idx24[:, :K])
    pt = ptp.tile([P, NCHUNK], F32, tag="tp", name="idxtp")
    nc.tensor.transpose(out=pt[:K, :P], in_=idf[:], identity=ident)
    mf = sp.tile([K, P], F32, tag="mf")
    nc.scalar.copy(out=mf[:], in_=pt[:K, :P])
    m16 = sp.tile([K, P], mybir.dt.int16, tag="m16")
    nc.vector.tensor_copy(out=m16[:], in_=mf[:])
    T = sp.tile([16, K * 8], mybir.dt.int16, tag="Tidx")
    nc.sync.dma_start(out=T[:].rearrange("c f -> f c"),
                      in_=m16[:].rearrange("g p -> (g p)").rearrange("(f c) -> f c", c=16))
    return T


def build_program():
    nc = bacc.Bacc("TRN2", target_bir_lowering=False, debug=False, num_devices=1)

    posT = nc.dram_tensor("posT", [E, 3, N], F32, kind="ExternalInput").ap()
    w = {}
    for name, shape in WEIGHT_SPECS:
        dt = U32 if name.startswith("rb") else F32
        w[name] = nc.dram_tensor(name, shape, dt, kind="ExternalInput").ap()
    out_d = nc.dram_tensor("out", [5, E], F32, kind="ExternalOutput").ap()

    with TileContext(nc) as tc, ExitStack() as ctx:
        wp = ctx.enter_context(tc.tile_pool(name="wp", bufs=1))
        ev = ctx.enter_context(tc.tile_pool(name="ev", bufs=1))
        sp = ctx.enter_context(tc.tile_pool(name="sp", bufs=2))
        dbig = ctx.enter_context(tc.tile_pool(name="dbig", bufs=2, space="DRAM"))
        pnu = ctx.enter_context(tc.tile_pool(name="pnu", bufs=2, space="PSUM"))
        ptp = ctx.enter_context(tc.tile_pool(name="ptp", bufs=2, space="PSUM"))
        pmm = ctx.enter_context(tc.tile_pool(name="pmm", bufs=2, space="PSUM"))

        # ---- load weights
        ws = {}
        for name, shape in WEIGHT_SPECS:
            dt = U32 if name.startswith("rb") else F32
            ws[name] = wp.tile(shape, dt, tag=name, name="w_" + name)
            nc.sync.dma_start(out=ws[name][:], in_=w[name][:])

        # ---- shared tiles (events processed sequentially; per-event tiles
        # share one tag/slot so SBUF holds only one event's working set)
        HV = ev.tile([P, E * 8], F32, tag="HV")
        ones3 = ev.tile([3, 1], F32, tag="ones3")
        ones64 = ev.tile([64, 1], F32, tag="ones64")
        nc.gpsimd.memset(ones3[:], 1.0)
        nc.gpsimd.memset(ones64[:], 1.0)

        for e in range(E):
            T4 = ev.tile([4, N], F32, tag="T4", name=f"T4_{e}")
            NR1 = ev.tile([4, N], F32, tag="NR1", name=f"NR1_{e}")
            A1 = ev.tile([P, NB * 64], F32, tag="A1", name=f"A1_{e}")
            T65 = ev.tile([65, N], F32, tag="T65", name=f"T65_{e}")
            NR2 = ev.tile([65, N], F32, tag="NR2", name=f"NR2_{e}")
            A2 = ev.tile([P, NB * 128], F32, tag="A2", name=f"A2_{e}")
            X2T = ev.tile([P, N], F32, tag="X2T", name=f"X2T_{e}")
            B1d = dbig.tile([N, 64], F32, tag="B1d", name=f"B1d_{e}")
            B2d = dbig.tile([N, 128], F32, tag="B2d", name=f"B2d_{e}")

            # ---- conv1 prep
            nc.sync.dma_start(out=T4[0:3, :], in_=posT[e])
            nc.sync.dma_start(out=T4[3:4, :], in_=w["row1"][:])
            nc.sync.dma_start(out=NR1[0:3, :], in_=posT[e])
            # -|x|^2/2 row: square, PE ones-reduce, scaled copy
            xsq = sp.tile([3, N], F32, tag="xsq", bufs=1)
            nc.scalar.activation(out=xsq[:], in_=T4[0:3, :], func=AF.Square)
            srow = sp.tile([1, N], F32, tag="srow", bufs=1)
            for c in range(N // NCHUNK):
                ps = pmm.tile([1, NCHUNK], F32, tag="mm")
                nc.tensor.matmul(out=ps[:], lhsT=ones3[:],
                                 rhs=xsq[:, c * NCHUNK:(c + 1) * NCHUNK],
                                 start=True, stop=True)
                nc.scalar.activation(out=srow[:, c * NCHUNK:(c + 1) * NCHUNK],
                                     in_=ps[:], func=AF.Copy, scale=-0.5)
            nc.sync.dma_start(out=NR1[3:4, :], in_=srow[:])
            # A1 / B1
            for blk in range(NB):
                s = slice(blk * P, (blk + 1) * P)
                pa = pmm.tile([P, 64], F32, tag="mm")
                nc.tensor.matmul(out=pa[:], lhsT=T4[0:3, s], rhs=ws["rA1"][:],
                                 start=True, stop=True)
                nc.scalar.copy(out=A1[:, blk * 64:(blk + 1) * 64], in_=pa[:])
                pb = pmm.tile([P, 64], F32, tag="mm")
                nc.tensor.matmul(out=pb[:], lhsT=T4[0:4, s], rhs=ws["rB1"][:],
                                 start=True, stop=True)
                bs = sp.tile([P, 64], F32, tag="bst")
                nc.scalar.copy(out=bs[:], in_=pb[:])
                nc.sync.dma_start(out=B1d[s, :], in_=bs[:])

            # ---- conv1 blocks
            for blk in range(NB):
                s = slice(blk * P, (blk + 1) * P)
                nus = sp.tile([P, N], F32, tag="nus")
                for h in range(2):
                    ph = pnu.tile([P, N // 2], F32, tag="nu")
                    for c in range(2):
                        cs = slice((2 * h + c) * NCHUNK, (2 * h + c + 1) * NCHUNK)
                        nc.tensor.matmul(out=ph[:, c * NCHUNK:(c + 1) * NCHUNK],
                                         lhsT=T4[0:4, s], rhs=NR1[0:4, cs],
                                         start=True, stop=True)
                    nc.scalar.copy(out=nus[:, h * (N // 2):(h + 1) * (N // 2)],
                                   in_=ph[:])
                idx = _topk20_idx(nc, sp, nus)

                eg1 = sp.tile([P, K * 64], F32, tag="eg1")
                for j in range(K):
                    nc.gpsimd.indirect_dma_start(
                        out=eg1[:, j * 64:(j + 1) * 64], out_offset=None,
                        in_=B1d[:, :],
                        in_offset=IndirectOffsetOnAxis(ap=idx[:, j:j + 1], axis=0))
                a1b = A1[:, blk * 64:(blk + 1) * 64]
                v = eg1[:].rearrange("p (j c) -> p j c", j=K)
                nc.vector.tensor_tensor(
                    out=v, in0=v,
                    in1=a1b.unsqueeze(1).to_broadcast([P, K, 64]), op=ALU.add)
                nc.scalar.activation(out=eg1[:], in_=eg1[:], func=AF.Relu)

                # transpose j-pairs -> eT [128 rows=(parity,ch), 10*128 pts]
                eT = sp.tile([P, 10 * P], F32, tag="eT")
                for grp in range(3):  # 4+4+2 pairs per psum tile
                    npair = 4 if grp < 2 else 2
                    pt = ptp.tile([P, NCHUNK], F32, tag="tp")
                    for t in range(npair):
                        tt = grp * 4 + t
                        nc.tensor.transpose(
                            out=pt[:, t * P:(t + 1) * P],
                            in_=eg1[:, tt * P:(tt + 1) * P], identity=ws["ident"][:])
                    nc.scalar.copy(
                        out=eT[:, grp * NCHUNK:grp * NCHUNK + npair * P],
                        in_=pt[:, :npair * P])

                # L2 + relu(+bias), L3, fused max over j, bias+relu -> x1 block
                e2Ta = sp.tile([64, 10 * P], F32, tag="e2Ta", bufs=1)
                e2Tb = sp.tile([64, 10 * P], F32, tag="e2Tb", bufs=1)
                e2Th = (e2Ta, e2Tb)
                for hh in range(2):
                    rhs = eT[64 * hh:64 * hh + 64, :]
                    for c in range(3):
                        w_ = NCHUNK if c < 2 else 256
                        cs = slice(c * NCHUNK, c * NCHUNK + w_)
                        pl = pmm.tile([64, NCHUNK], F32, tag="mm")
                        nc.tensor.matmul(out=pl[:, :w_],
                                         lhsT=ws["W2f"][64 * hh:64 * hh + 64, :],
                                         rhs=rhs[:, cs], start=True, stop=True)
                        nc.scalar.activation(out=e2Th[hh][:, cs],
                                             in_=pl[:, :w_], func=AF.Relu,
                                             bias=ws["b2f"][:])
                x1s = T65[0:64, s]
                first = True
                for hh in range(2):
                    rhs = e2Th[hh][:]
                    for c in range(3):
                        w_ = NCHUNK if c < 2 else 256
                        nt = w_ // P
                        cs = slice(c * NCHUNK, c * NCHUNK + w_)
                        pl = pmm.tile([64, NCHUNK], F32, tag="mm")
                        nc.tensor.matmul(out=pl[:, :w_], lhsT=ws["W3f"][:],
                                         rhs=rhs[:, cs], start=True, stop=True)
                        part = sp.tile([64, P], F32, tag="part")
                        nc.vector.tensor_reduce(
                            out=part[:],
                            in_=pl[:, :w_].rearrange("m (t p) -> m p t", t=nt),
                            axis=AX.X, op=ALU.max)
                        if first:
                            nc.vector.tensor_copy(out=x1s, in_=part[:])
                            first = False
                        else:
                            nc.vector.tensor_tensor(out=x1s, in0=x1s, in1=part[:],
                                                    op=ALU.max)
                nc.scalar.activation(out=x1s, in_=x1s, func=AF.Relu,
                                     bias=ws["b3f"][:])

            # ---- conv2 prep
            # x1 affine (BN of conv1 layer 3)
            nc.vector.tensor_scalar(out=T65[0:64, :], in0=T65[0:64, :],
                                    scalar1=ws["g3"][:], scalar2=ws["be3"][:],
                                    op0=ALU.mult, op1=ALU.add)
            nc.sync.dma_start(out=T65[64:65, :], in_=w["row1"][:])
            nc.scalar.copy(out=NR2[0:64, :], in_=T65[0:64, :])
            x1sq = sp.tile([64, N], F32, tag="nus", name=f"x1sq_{e}")
            nc.scalar.activation(out=x1sq[:], in_=T65[0:64, :], func=AF.Square)
            srow2 = sp.tile([1, N], F32, tag="srow", bufs=1, name=f"srow2_{e}")
            for c in range(N // NCHUNK):
                ps = pmm.tile([1, NCHUNK], F32, tag="mm")
                nc.tensor.matmul(out=ps[:], lhsT=ones64[:],
                                 rhs=x1sq[:, c * NCHUNK:(c + 1) * NCHUNK],
                                 start=True, stop=True)
                nc.scalar.activation(out=srow2[:, c * NCHUNK:(c + 1) * NCHUNK],
                                     in_=ps[:], func=AF.Copy, scale=-0.5)
            nc.sync.dma_start(out=NR2[64:65, :], in_=srow2[:])
            for blk in range(NB):
                s = slice(blk * P, (blk + 1) * P)
                pa = pmm.tile([P, 128], F32, tag="mm")
                nc.tensor.matmul(out=pa[:], lhsT=T65[0:64, s], rhs=ws["rA2"][:],
                                 start=True, stop=True)
                nc.scalar.copy(out=A2[:, blk * 128:(blk + 1) * 128], in_=pa[:])
                pb = pmm.tile([P, 128], F32, tag="mm")
                nc.tensor.matmul(out=pb[:], lhsT=T65[0:65, s], rhs=ws["rB2"][:],
                                 start=True, stop=True)
                bs = sp.tile([P, 128], F32, tag="bst2")
                nc.scalar.copy(out=bs[:], in_=pb[:])
                nc.sync.dma_start(out=B2d[s, :], in_=bs[:])

            # ---- conv2 blocks
            for blk in range(NB):
                s = slice(blk * P, (blk + 1) * P)
                nus = sp.tile([P, N], F32, tag="nus")
                for h in range(2):
                    ph = pnu.tile([P, N // 2], F32, tag="nu")
                    for c in range(2):
                        cs = slice((2 * h + c) * NCHUNK, (2 * h + c + 1) * NCHUNK)
                        nc.tensor.matmul(out=ph[:, c * NCHUNK:(c + 1) * NCHUNK],
                                         lhsT=T65[0:65, s], rhs=NR2[0:65, cs],
                                         start=True, stop=True)
                    nc.scalar.copy(out=nus[:, h * (N // 2):(h + 1) * (N // 2)],
                                   in_=ph[:])
                idx = _topk20_idx(nc, sp, nus)

                eg2 = sp.tile([P, K * 128], F32, tag="eg2")
                for j in range(K):
                    nc.gpsimd.indirect_dma_start(
                        out=eg2[:, j * 128:(j + 1) * 128], out_offset=None,
                        in_=B2d[:, :],
                        in_offset=IndirectOffsetOnAxis(ap=idx[:, j:j + 1], axis=0))
                # max over the 20 neighbors: binary tree
                gA = sp.tile([P, 10 * 128], F32, tag="eg1", name=f"gA_{e}_{blk}")
                nc.vector.tensor_tensor(out=gA[:], in0=eg2[:, :1280],
                                        in1=eg2[:, 1280:], op=ALU.max)
                t5 = sp.tile([P, 5 * 128], F32, tag="t5", bufs=1)
                nc.vector.tensor_tensor(out=t5[:], in0=gA[:, :640],
                                        in1=gA[:, 640:], op=ALU.max)
                x2b = sp.tile([P, 128], F32, tag="x2b")
                t2 = sp.tile([P, 2 * 128], F32, tag="t2")
                nc.vector.tensor_tensor(out=t2[:], in0=t5[:, :256],
                                        in1=t5[:, 256:512], op=ALU.max)
                nc.vector.tensor_tensor(out=t2[:, :128], in0=t2[:, :128],
                                        in1=t2[:, 128:], op=ALU.max)
                nc.vector.tensor_tensor(out=x2b[:], in0=t2[:, :128],
                                        in1=t5[:, 512:], op=ALU.max)
                nc.vector.tensor_tensor(out=x2b[:], in0=x2b[:],
                                        in1=A2[:, blk * 128:(blk + 1) * 128],
                                        op=ALU.add)
                nc.scalar.activation(out=x2b[:], in_=x2b[:], func=AF.Relu)
                pt = ptp.tile([P, NCHUNK], F32, tag="tp")
                nc.tensor.transpose(out=pt[:, :P], in_=x2b[:],
                                    identity=ws["ident"][:])
                nc.scalar.copy(out=X2T[:, s], in_=pt[:, :P])

            # ---- lin1 + global max pool (this event)
            nc.vector.tensor_scalar(out=X2T[:], in0=X2T[:],
                                    scalar1=ws["gc2"][:], scalar2=ws["bec2"][:],
                                    op0=ALU.mult, op1=ALU.add)
            # HV column layout: m * E + e (chunk k of mlp1 = HV[:, k*E:(k+1)*E])
            for m in range(8):
                ms = slice(m * 128, (m + 1) * 128)
                zm4 = sp.tile([P, 4], F32, tag="zm4")
                for c in range(4):
                    cs = slice(c * NCHUNK, (c + 1) * NCHUNK)
                    pl = pmm.tile([P, NCHUNK], F32, tag="mm")
                    nc.tensor.matmul(out=pl[:], lhsT=ws["Wl1a"][:, ms],
                                     rhs=T65[0:64, cs], start=True, stop=False)
                    nc.tensor.matmul(out=pl[:], lhsT=ws["Wl1b"][:, ms],
                                     rhs=X2T[:, cs], start=False, stop=True)
                    nc.vector.tensor_reduce(out=zm4[:, c:c + 1], in_=pl[:],
                                            axis=AX.X, op=ALU.max)
                nc.vector.tensor_reduce(out=HV[:, m * E + e:m * E + e + 1],
                                        in_=zm4[:], axis=AX.X, op=ALU.max)
            hve = HV[:].rearrange("p (m e) -> p m e", e=E)[:, :, e:e + 1]
            nc.vector.tensor_tensor(out=hve, in0=hve,
                                    in1=ws["bl1"][:].unsqueeze(2), op=ALU.add)
            nc.scalar.activation(out=hve, in_=hve, func=AF.Relu)

        R1 = ev.tile([P, 8], F32, tag="R1")
        for m in range(4):
            pl = pmm.tile([P, E], F32, tag="mm")
            for k in range(8):
                lh = ws["Wm1s"][:, k * NCHUNK + m * 128:k * NCHUNK + (m + 1) * 128]
                nc.tensor.matmul(out=pl[:], lhsT=lh, rhs=HV[:, k * E:(k + 1) * E],
                                 start=(k == 0), stop=(k == 7))
            nc.scalar.activation(out=R1[:, m * E:(m + 1) * E], in_=pl[:],
                                 func=AF.Relu, bias=ws["bm1f"][:, m:m + 1])
        R2 = ev.tile([P, 4], F32, tag="R2")
        for m in range(2):
            pl = pmm.tile([P, E], F32, tag="mm")
            for k in range(4):
                lh = ws["Wm2s"][:, k * 256 + m * 128:k * 256 + (m + 1) * 128]
                nc.tensor.matmul(out=pl[:], lhsT=lh, rhs=R1[:, k * E:(k + 1) * E],
                                 start=(k == 0), stop=(k == 3))
            nc.scalar.activation(out=R2[:, m * E:(m + 1) * E], in_=pl[:],
                                 func=AF.Relu, bias=ws["bm2f"][:, m:m + 1])
        po = pmm.tile([5, E], F32, tag="mm")
        for k in range(2):
            nc.tensor.matmul(out=po[:], lhsT=ws["Whs"][:, k * 5:(k + 1) * 5],
                             rhs=R2[:, k * E:(k + 1) * E],
                             start=(k == 0), stop=(k == 1))
        fin = ev.tile([5, E], F32, tag="fin")
        nc.scalar.activation(out=fin[:], in_=po[:], func=AF.Identity,
                             bias=ws["bhf"][:])
        nc.sync.dma_start(out=out_d[:, :], in_=fin[:])

    nc.compile()
    return nc


def make_in_maps(pos, params):
    pos = _np(pos).astype(np.float32).reshape(B, N, 3)
    wts = fold_params(params)
    ident = np.eye(128, dtype=np.float32)
    base = dict(wts, ident=ident, row1=np.ones((1, N), dtype=np.float32))
    in_maps = []
    for c in range(NCORES):
        m = dict(base)
        m["posT"] = np.ascontiguousarray(
            pos[c * E:(c + 1) * E].transpose(0, 2, 1))  # [E, 3, N]
        in_maps.append(m)
    return in_maps


_NC_CACHE = None


def kernel(pos, params, batch):
    global _NC_CACHE
    from concourse.bass_utils import run_bass_kernel_spmd
    if _NC_CACHE is None:
        _NC_CACHE = build_program()
    nc = _NC_CACHE
    in_maps = make_in_maps(pos, params)
    res = run_bass_kernel_spmd(nc, in_maps, core_ids=list(range(NCORES)))
    outs = [res.results[c]["out"].T for c in range(NCORES)]  # each [E, 5]
    return np.concatenate(outs, axis=0).astype(np.float32)
